# revision 1
# baseline (speedup 1.0000x reference)
"""GRU (Flax GRUCell scanned over time) on 8 Trainium2 NeuronCores.

Problem: x:[T,B,D]=[512,64,512], h0:[B,H], Wi:[D,3H], Wh:[H,3H], bi:[3H], bhn:[H]
  gi = x_t @ Wi + bi ; gh = h @ Wh ; gates (r,z,n); h' = (1-z)*n + z*h
  returns ys:[T,B,H] (the h trajectory).

Strategy (per core, data-parallel over batch, B_local=8):
  Everything on-chip lives in "T-layout": hidden dim on SBUF partitions,
  batch on the free dim, so elementwise work uses all 128 lanes.
  Per step, two PSUM accumulations with stationary bf16 weights:
    - ghT = Wh.T @ hT (on the critical chain: needs h from last step)
    - giT(t+1) = Wi.T @ xT(t+1) (+bi) — independent of the recurrence, these
      matmuls fill the PE idle window while the gate elementwise chain runs.
  Gate math: 7-op critical chain
    pre_rz -> sigmoid -> rpn -> pre_n -> tanh -> w -> hb(bf16)
  with omz = 1-z, v = z*h, and the fp32 h' kept off-chain on GpSimd.
  h state stays fp32; output is written transposed and reassembled on host.
"""

import warnings

warnings.filterwarnings("ignore")

import numpy as np
import ml_dtypes

import concourse.bacc as bacc
import concourse.tile as tile
from concourse import mybir, bass_utils

B, D, H = 64, 512, 512
NCORES = 8
BL = B // NCORES  # batch per core
KD = D // 128  # input-dim k-chunks
KH = H // 128  # hidden-dim k-chunks
M3 = 3 * H // 128  # 3H m-tiles
RT = 8  # output-ring steps per DMA
BF16 = mybir.dt.bfloat16
F32 = mybir.dt.float32
NPBF16 = ml_dtypes.bfloat16

_cache: dict = {}


def _build(T: int, use_bi: bool, use_bhn: bool):
    TB = T * BL
    assert T % RT == 0
    nc = bacc.Bacc("TRN2", target_bir_lowering=False, debug=False, num_devices=NCORES)

    xt_d = nc.dram_tensor("xt", [128, KD * TB], BF16, kind="ExternalInput").ap()
    wi_d = nc.dram_tensor("wi", [128, M3 * KD * 128], BF16, kind="ExternalInput").ap()
    wh_d = nc.dram_tensor("wh", [128, M3 * KH * 128], BF16, kind="ExternalInput").ap()
    h0_d = nc.dram_tensor("h0t", [128, KH * BL], F32, kind="ExternalInput").ap()
    bi_d = (
        nc.dram_tensor("bi_r", [1, M3 * 128], BF16, kind="ExternalInput").ap()
        if use_bi
        else None
    )
    bhn_d = (
        nc.dram_tensor("bhn_t", [128, KH], F32, kind="ExternalInput").ap()
        if use_bhn
        else None
    )
    ys_d = nc.dram_tensor("yst", [128, KH * TB], F32, kind="ExternalOutput").ap()
    ys_v = ys_d.rearrange("p (k t j) -> p k t j", k=KH, j=BL)

    with tile.TileContext(nc) as tc:
        with (
            tc.tile_pool(name="const", bufs=1) as const,
            tc.tile_pool(name="xin", bufs=1) as xin,
            tc.tile_pool(name="whps", bufs=2, space="PSUM") as whps,
            tc.tile_pool(name="gips", bufs=2, space="PSUM") as gips,
            tc.tile_pool(name="orp", bufs=3) as orp,
            tc.tile_pool(name="hbp", bufs=2) as hbp,
            tc.tile_pool(name="ew", bufs=2) as ew,
        ):
            # ---- load constants ----
            wi_sb = const.tile([128, M3 * KD * 128], BF16)
            nc.sync.dma_start(wi_sb[:], wi_d[:])
            wh_sb = const.tile([128, M3 * KH * 128], BF16)
            nc.sync.dma_start(wh_sb[:], wh_d[:])
            h0_sb = const.tile([128, KH, BL], F32)
            nc.sync.dma_start(h0_sb[:], h0_d.rearrange("p (k j) -> p k j", j=BL))
            if use_bi:
                bi_sb = const.tile([1, M3 * 128], BF16)
                nc.sync.dma_start(bi_sb[:], bi_d[:])
                ones_sb = const.tile([1, BL], BF16)
                nc.vector.memset(ones_sb[:], 1.0)
            if use_bhn:
                bhn_sb = const.tile([128, KH], F32)
                nc.sync.dma_start(bhn_sb[:], bhn_d[:])
            xt_sb = xin.tile([128, KD * TB], BF16)
            nc.sync.dma_start(xt_sb[:], xt_d[:])

            def gi_mms(t):
                """giT(t) (+bi) into a fresh PSUM tile; PE-idle filler work.
                The r/z regions (m 0..7) are left open: the next step's Wh
                matmuls accumulate gh on top, so sigmoid reads gh+gi straight
                from PSUM."""
                gp = gips.tile([128, M3, BL], F32, tag="gip")
                for m in range(M3):
                    last_gi = (m >= 8) and not use_bi
                    for k in range(KD):
                        nc.tensor.matmul(
                            gp[:, m, :],
                            wi_sb[:, (m * KD + k) * 128 : (m * KD + k + 1) * 128],
                            xt_sb[:, k * TB + t * BL : k * TB + t * BL + BL],
                            # start=True lazily zeroes the whole 2KB psum
                            # zero-region (the bank), so only the very first
                            # matmul of this tile may set it.
                            start=(m == 0 and k == 0),
                            stop=(k == KD - 1) and last_gi,
                            skip_group_check=True,
                        )
                    if use_bi:
                        nc.tensor.matmul(
                            gp[:, m, :],
                            bi_sb[:, m * 128 : (m + 1) * 128],
                            ones_sb[:],
                            start=False,
                            stop=(m >= 8),
                            skip_group_check=True,
                        )
                return gp

            hb = hbp.tile([128, KH, BL], BF16, tag="hb")
            nc.vector.tensor_copy(hb[:], h0_sb[:])
            h_prev = h0_sb[:, :, :]
            gp = gi_mms(0)

            o_cur = None
            for t in range(T):
                u = t % RT
                if u == 0:
                    o_cur = orp.tile([128, KH, RT, BL], F32, tag="oring")

                # on-chain: ghT matmuls (need h from last step).
                # r/z rows accumulate into gp (on top of gi); n rows into
                # their own psum so r can gate gh_n alone.
                ps = whps.tile([128, KH, BL], F32, tag="whp")
                for k in range(KH):  # k-outer: each pass needs only hb[:,k,:]
                    for m in range(M3):
                        out_ap = gp[:, m, :] if m < 8 else ps[:, m - 8, :]
                        nc.tensor.matmul(
                            out_ap,
                            wh_sb[:, (m * KH + k) * 128 : (m * KH + k + 1) * 128],
                            hb[:, k, :],
                            start=(m == 8 and k == 0),
                            stop=(k == KH - 1),
                            skip_group_check=True,
                        )
                # off-chain: next step's giT fills the PE idle window
                gp_next = gi_mms(t + 1) if t + 1 < T else None

                # Gate math.  Critical chain (6 ops):
                #   sigmoid(psum) -> rpn -> pre_n -> tanh -> w -> hb
                # Off-chain on GpSimd: omz = 1-z, v = z*h_prev, fp32 h'.
                # h' = (1-z)*n + z*h = omz*n + v
                rzt = ew.tile([128, 8, BL], F32, tag="rzt")
                nc.scalar.activation(
                    rzt[:], gp[:, 0:8, :], mybir.ActivationFunctionType.Sigmoid
                )
                omz = ew.tile([128, KH, BL], F32, tag="omz")
                nc.gpsimd.tensor_scalar(
                    omz[:],
                    rzt[:, KH : 2 * KH, :],
                    -1.0,
                    1.0,
                    mybir.AluOpType.mult,
                    mybir.AluOpType.add,
                )
                v = ew.tile([128, KH, BL], F32, tag="v")
                nc.gpsimd.tensor_mul(v[:], rzt[:, KH : 2 * KH, :], h_prev)
                rpn = ew.tile([128, KH, BL], F32, tag="rpn")
                if use_bhn:
                    for k in range(KH):
                        nc.vector.scalar_tensor_tensor(
                            rpn[:, k, :],
                            ps[:, k, :],
                            bhn_sb[:, k : k + 1],
                            rzt[:, k, :],
                            mybir.AluOpType.add,
                            mybir.AluOpType.mult,
                        )
                else:
                    nc.vector.tensor_mul(rpn[:], ps[:], rzt[:, 0:KH, :])
                pre_n = ew.tile([128, KH, BL], F32, tag="pren")
                nc.vector.tensor_add(pre_n[:], rpn[:], gp[:, 8:12, :])
                nt = ew.tile([128, KH, BL], F32, tag="nt")
                nc.scalar.activation(
                    nt[:], pre_n[:], mybir.ActivationFunctionType.Tanh
                )
                w = ew.tile([128, KH, BL], F32, tag="w")
                hb = hbp.tile([128, KH, BL], BF16, tag="hb")
                # per-chunk tail: hb[:,k,:] unblocks the next step's k-th
                # matmul pass while later chunks are still in flight
                for k in range(KH):
                    nc.vector.tensor_mul(w[:, k, :], nt[:, k, :], omz[:, k, :])
                    nc.vector.tensor_add(hb[:, k, :], w[:, k, :], v[:, k, :])
                h_new = o_cur[:, :, u, :]
                # fp32 h for output/next-step v, off the critical chain
                nc.gpsimd.tensor_add(h_new, w[:], v[:])
                h_prev = h_new
                gp = gp_next

                if u == RT - 1:
                    nc.sync.dma_start(
                        ys_v[:, :, t - RT + 1 : t + 1, :], o_cur[:]
                    )

    nc.compile()
    return nc


def _get(T, use_bi, use_bhn):
    key = (T, use_bi, use_bhn)
    if key not in _cache:
        _cache[key] = _build(T, use_bi, use_bhn)
    return _cache[key]


def _pack_w(W, kc):
    # W [kc*128, M3*128] -> [128, M3*kc*128], col ((m*kc)+k)*128+c = W[k*128+p, m*128+c]
    return np.ascontiguousarray(
        W.astype(NPBF16).reshape(kc, 128, M3, 128).transpose(1, 2, 0, 3).reshape(128, -1)
    )


def kernel(x, h0, Wi, Wh, bi, bhn, _trace=False, _trace_kwargs=None):
    T = x.shape[0]
    use_bi = bool(np.any(bi))
    use_bhn = bool(np.any(bhn))
    nc = _get(T, use_bi, use_bhn)
    TB = T * BL

    wi_p = _pack_w(np.asarray(Wi), KD)
    wh_p = _pack_w(np.asarray(Wh), KH)
    x = np.asarray(x)
    h0 = np.asarray(h0)

    in_maps = []
    for c in range(NCORES):
        xc = x[:, c * BL : (c + 1) * BL, :]  # [T, BL, D]
        xt = np.ascontiguousarray(
            xc.astype(NPBF16).reshape(T, BL, KD, 128).transpose(3, 2, 0, 1).reshape(128, KD * TB)
        )
        h0c = np.ascontiguousarray(
            h0[c * BL : (c + 1) * BL, :].astype(np.float32).reshape(BL, KH, 128).transpose(2, 1, 0).reshape(128, KH * BL)
        )
        im = {"xt": xt, "wi": wi_p, "wh": wh_p, "h0t": h0c}
        if use_bi:
            im["bi_r"] = np.ascontiguousarray(bi.astype(NPBF16).reshape(1, M3 * 128))
        if use_bhn:
            im["bhn_t"] = np.ascontiguousarray(bhn.astype(np.float32).reshape(KH, 128).T)
        in_maps.append(im)

    kw = {}
    if _trace:
        kw = dict(trace=True, **(_trace_kwargs or {}))
    kernel._last_in_maps = in_maps
    res = bass_utils.run_bass_kernel_spmd(nc, in_maps, core_ids=list(range(NCORES)), **kw)

    ys = np.empty((T, B, H), dtype=np.float32)
    for c in range(NCORES):
        out = res.results[c]["yst"]  # [128, KH*TB]
        ys[:, c * BL : (c + 1) * BL, :] = (
            out.reshape(128, KH, T, BL).transpose(2, 3, 1, 0).reshape(T, BL, H)
        )
    kernel._last_result = res
    return ys



# revision 2
# speedup vs baseline: 1.0385x; 1.0385x over previous
"""GRU (Flax GRUCell scanned over time) on 8 Trainium2 NeuronCores.

Problem: x:[T,B,D]=[512,64,512], h0:[B,H], Wi:[D,3H], Wh:[H,3H], bi:[3H], bhn:[H]
  gi = x_t @ Wi + bi ; gh = h @ Wh ; gates (r,z,n); h' = (1-z)*n + z*h
  returns ys:[T,B,H] (the h trajectory).

Strategy (per core, data-parallel over batch, B_local=8):
  Everything on-chip lives in "T-layout": hidden dim on SBUF partitions,
  batch on the free dim, so elementwise work uses all 128 lanes.
  Per step, two PSUM accumulations with stationary bf16 weights:
    - ghT = Wh.T @ hT (on the critical chain: needs h from last step).
      The r/z m-tiles are issued FIRST (m-outer) so their accumulation
      groups close early: sigmoid's ~750ns PE-sem wait overlaps the
      n-part matmuls instead of extending the chain.
    - giT(t+1) = Wi.T @ xT(t+1) (+bi) — independent of the recurrence,
      fills the PE idle window while the gate elementwise chain runs.
  Gate math critical chain (whole-tile ops):
    sigmoid(psum) -> rpn -> pre_n -> tanh -> w -> hb(bf16)
  with omz = 1-z, v = z*h, and the fp32 h' kept off-chain on GpSimd.
  h state stays fp32; output is written transposed and reassembled on host.
"""

import warnings

warnings.filterwarnings("ignore")

import numpy as np
import ml_dtypes

import concourse.bacc as bacc
import concourse.tile as tile
from concourse import mybir, bass_utils

B, D, H = 64, 512, 512
NCORES = 8
BL = B // NCORES  # batch per core
KD = D // 128  # input-dim k-chunks
KH = H // 128  # hidden-dim k-chunks
M3 = 3 * H // 128  # 3H m-tiles
RT = 8  # output-ring steps per DMA
BF16 = mybir.dt.bfloat16
F32 = mybir.dt.float32
NPBF16 = ml_dtypes.bfloat16

_cache: dict = {}


def _build(T: int, use_bi: bool, use_bhn: bool):
    TB = T * BL
    assert T % RT == 0
    nc = bacc.Bacc("TRN2", target_bir_lowering=False, debug=False, num_devices=NCORES)

    xt_d = nc.dram_tensor("xt", [128, KD * TB], BF16, kind="ExternalInput").ap()
    wi_d = nc.dram_tensor("wi", [128, M3 * KD * 128], BF16, kind="ExternalInput").ap()
    wh_d = nc.dram_tensor("wh", [128, M3 * KH * 128], BF16, kind="ExternalInput").ap()
    h0_d = nc.dram_tensor("h0t", [128, KH * BL], F32, kind="ExternalInput").ap()
    bi_d = (
        nc.dram_tensor("bi_r", [1, M3 * 128], BF16, kind="ExternalInput").ap()
        if use_bi
        else None
    )
    bhn_d = (
        nc.dram_tensor("bhn_t", [128, KH], F32, kind="ExternalInput").ap()
        if use_bhn
        else None
    )
    ys_d = nc.dram_tensor("yst", [128, KH * TB], F32, kind="ExternalOutput").ap()
    ys_v = ys_d.rearrange("p (k t j) -> p k t j", k=KH, j=BL)

    with tile.TileContext(nc) as tc:
        with (
            tc.tile_pool(name="const", bufs=1) as const,
            tc.tile_pool(name="xin", bufs=1) as xin,
            tc.tile_pool(name="whps", bufs=3, space="PSUM") as whps,
            tc.tile_pool(name="gips", bufs=3, space="PSUM") as gips,
            tc.tile_pool(name="orp", bufs=3) as orp,
            tc.tile_pool(name="hbp", bufs=3) as hbp,
            tc.tile_pool(name="ew", bufs=2) as ew,
        ):
            # ---- load constants ----
            wi_sb = const.tile([128, M3 * KD * 128], BF16)
            nc.sync.dma_start(wi_sb[:], wi_d[:])
            wh_sb = const.tile([128, M3 * KH * 128], BF16)
            nc.sync.dma_start(wh_sb[:], wh_d[:])
            h0_sb = const.tile([128, KH, BL], F32)
            nc.sync.dma_start(h0_sb[:], h0_d.rearrange("p (k j) -> p k j", j=BL))
            if use_bi:
                bi_sb = const.tile([1, M3 * 128], BF16)
                nc.sync.dma_start(bi_sb[:], bi_d[:])
                ones_sb = const.tile([1, BL], BF16)
                nc.vector.memset(ones_sb[:], 1.0)
            if use_bhn:
                bhn_sb = const.tile([128, KH], F32)
                nc.sync.dma_start(bhn_sb[:], bhn_d[:])
            xt_sb = xin.tile([128, KD * TB], BF16)
            nc.sync.dma_start(xt_sb[:], xt_d[:])

            def gi_mms(t):
                """giT(t) (+bi) into a fresh PSUM tile; PE-idle filler work.
                The r/z regions (m 0..7) are left open: the next step's Wh
                matmuls accumulate gh on top, so sigmoid reads gh+gi straight
                from PSUM."""
                gp = gips.tile([128, M3, BL], F32, tag="gip")
                for m in range(M3):
                    last_gi = (m >= 8) and not use_bi
                    for k in range(KD):
                        nc.tensor.matmul(
                            gp[:, m, :],
                            wi_sb[:, (m * KD + k) * 128 : (m * KD + k + 1) * 128],
                            xt_sb[:, k * TB + t * BL : k * TB + t * BL + BL],
                            # start=True lazily zeroes the whole 2KB psum
                            # zero-region (the bank), so only the very first
                            # matmul of this tile may set it.
                            start=(m == 0 and k == 0),
                            stop=(k == KD - 1) and last_gi,
                            skip_group_check=True,
                        )
                    if use_bi:
                        nc.tensor.matmul(
                            gp[:, m, :],
                            bi_sb[:, m * 128 : (m + 1) * 128],
                            ones_sb[:],
                            start=False,
                            stop=(m >= 8),
                            skip_group_check=True,
                        )
                return gp

            hb = hbp.tile([128, KH, BL], BF16, tag="hb")
            nc.vector.tensor_copy(hb[:], h0_sb[:])
            h_prev = h0_sb[:, :, :]
            gp = gi_mms(0)

            o_cur = None
            for t in range(T):
                u = t % RT
                if u == 0:
                    o_cur = orp.tile([128, KH, RT, BL], F32, tag="oring")

                # on-chain: ghT matmuls (need h from last step).
                # r/z rows accumulate into gp (on top of gi); n rows into
                # their own psum so r can gate gh_n alone.
                # m-outer, r/z first: their groups close ~430ns before the
                # n rows, hiding sigmoid's PE-sem wait behind the n matmuls.
                ps = whps.tile([128, KH, BL], F32, tag="whp")
                for m in range(M3):
                    out_ap = gp[:, m, :] if m < 8 else ps[:, m - 8, :]
                    for k in range(KH):
                        nc.tensor.matmul(
                            out_ap,
                            wh_sb[:, (m * KH + k) * 128 : (m * KH + k + 1) * 128],
                            hb[:, k, :],
                            start=(m == 8 and k == 0),
                            stop=(k == KH - 1),
                            skip_group_check=True,
                        )
                # off-chain: next step's giT fills the PE idle window
                gp_next = gi_mms(t + 1) if t + 1 < T else None

                # Gate math.  Critical chain (whole-tile ops):
                #   sigmoid(psum) -> rpn -> pre_n -> tanh -> w -> hb
                # Off-chain on GpSimd: omz = 1-z, v = z*h_prev, fp32 h'.
                # h' = (1-z)*n + z*h = omz*n + v
                rzt = ew.tile([128, 8, BL], F32, tag="rzt")
                nc.scalar.activation(
                    rzt[:], gp[:, 0:8, :], mybir.ActivationFunctionType.Sigmoid
                )
                omz = ew.tile([128, KH, BL], F32, tag="omz")
                nc.gpsimd.tensor_scalar(
                    omz[:],
                    rzt[:, KH : 2 * KH, :],
                    -1.0,
                    1.0,
                    mybir.AluOpType.mult,
                    mybir.AluOpType.add,
                )
                v = ew.tile([128, KH, BL], F32, tag="v")
                nc.gpsimd.tensor_mul(v[:], rzt[:, KH : 2 * KH, :], h_prev)
                rpn = ew.tile([128, KH, BL], F32, tag="rpn")
                if use_bhn:
                    for k in range(KH):
                        nc.vector.scalar_tensor_tensor(
                            rpn[:, k, :],
                            ps[:, k, :],
                            bhn_sb[:, k : k + 1],
                            rzt[:, k, :],
                            mybir.AluOpType.add,
                            mybir.AluOpType.mult,
                        )
                else:
                    nc.vector.tensor_mul(rpn[:], ps[:], rzt[:, 0:KH, :])
                pre_n = ew.tile([128, KH, BL], F32, tag="pren")
                nc.vector.tensor_add(pre_n[:], rpn[:], gp[:, 8:12, :])
                nt = ew.tile([128, KH, BL], F32, tag="nt")
                nc.scalar.activation(
                    nt[:], pre_n[:], mybir.ActivationFunctionType.Tanh
                )
                w = ew.tile([128, KH, BL], F32, tag="w")
                hb = hbp.tile([128, KH, BL], BF16, tag="hb")
                nc.vector.tensor_mul(w[:], nt[:], omz[:])
                nc.vector.tensor_add(hb[:], w[:], v[:])
                h_new = o_cur[:, :, u, :]
                # fp32 h for output/next-step v, off the critical chain
                nc.gpsimd.tensor_add(h_new, w[:], v[:])
                h_prev = h_new
                gp = gp_next

                if u == RT - 1:
                    nc.sync.dma_start(
                        ys_v[:, :, t - RT + 1 : t + 1, :], o_cur[:]
                    )

    nc.compile()
    return nc


def _get(T, use_bi, use_bhn):
    key = (T, use_bi, use_bhn)
    if key not in _cache:
        _cache[key] = _build(T, use_bi, use_bhn)
    return _cache[key]


def _pack_w(W, kc):
    # W [kc*128, M3*128] -> [128, M3*kc*128], col ((m*kc)+k)*128+c = W[k*128+p, m*128+c]
    return np.ascontiguousarray(
        W.astype(NPBF16).reshape(kc, 128, M3, 128).transpose(1, 2, 0, 3).reshape(128, -1)
    )


def kernel(x, h0, Wi, Wh, bi, bhn, _trace=False, _trace_kwargs=None):
    T = x.shape[0]
    use_bi = bool(np.any(bi))
    use_bhn = bool(np.any(bhn))
    nc = _get(T, use_bi, use_bhn)
    TB = T * BL

    wi_p = _pack_w(np.asarray(Wi), KD)
    wh_p = _pack_w(np.asarray(Wh), KH)
    x = np.asarray(x)
    h0 = np.asarray(h0)

    in_maps = []
    for c in range(NCORES):
        xc = x[:, c * BL : (c + 1) * BL, :]  # [T, BL, D]
        xt = np.ascontiguousarray(
            xc.astype(NPBF16).reshape(T, BL, KD, 128).transpose(3, 2, 0, 1).reshape(128, KD * TB)
        )
        h0c = np.ascontiguousarray(
            h0[c * BL : (c + 1) * BL, :].astype(np.float32).reshape(BL, KH, 128).transpose(2, 1, 0).reshape(128, KH * BL)
        )
        im = {"xt": xt, "wi": wi_p, "wh": wh_p, "h0t": h0c}
        if use_bi:
            im["bi_r"] = np.ascontiguousarray(bi.astype(NPBF16).reshape(1, M3 * 128))
        if use_bhn:
            im["bhn_t"] = np.ascontiguousarray(bhn.astype(np.float32).reshape(KH, 128).T)
        in_maps.append(im)

    kw = {}
    if _trace:
        kw = dict(trace=True, **(_trace_kwargs or {}))
    kernel._last_in_maps = in_maps
    res = bass_utils.run_bass_kernel_spmd(nc, in_maps, core_ids=list(range(NCORES)), **kw)

    ys = np.empty((T, B, H), dtype=np.float32)
    for c in range(NCORES):
        out = res.results[c]["yst"]  # [128, KH*TB]
        ys[:, c * BL : (c + 1) * BL, :] = (
            out.reshape(128, KH, T, BL).transpose(2, 3, 1, 0).reshape(T, BL, H)
        )
    kernel._last_result = res
    return ys


# revision 3
# speedup vs baseline: 1.1717x; 1.1283x over previous
"""GRU (Flax GRUCell scanned over time) on 8 Trainium2 NeuronCores.

Problem: x:[T,B,D]=[512,64,512], h0:[B,H], Wi:[D,3H], Wh:[H,3H], bi:[3H], bhn:[H]
  gi = x_t @ Wi + bi ; gh = h @ Wh ; gates (r,z,n); h' = (1-z)*n + z*h
  returns ys:[T,B,H] (the h trajectory).

Strategy (per core, data-parallel over batch, B_local=8):
  Everything on-chip lives in "T-layout": hidden dim on SBUF partitions,
  batch on the free dim.

  Phase 1 (one-time, ~90us): gi = x @ Wi for ALL T steps as dense
  N=512 matmuls (weight loads amortized 64x vs per-step), stored bf16
  in SBUF ([128, M3, T*BL] ~ 98KB/partition).

  Phase 2 (the scan): per step only the 48 gh matmuls run on the PE.
    - the r/z psum bank is PREFILLED with gi_rz by an off-chain DVE
      copy (2 steps ahead); gh r/z matmuls accumulate on top with
      start=False, so sigmoid still reads (gi+gh) straight from PSUM.
    - r/z m-tiles issue first (m-outer) so their accumulation groups
      close early; sigmoid's PE-sem wait overlaps the n-part matmuls.
    - gi_n is read from SBUF by pre_n (it cannot be pre-merged into
      the gh_n psum because r gates only gh_n).
  Gate math critical chain (whole-tile ops):
    sigmoid(psum) -> rpn -> pre_n -> tanh -> w -> hb(bf16)
  with omz = 1-z, v = z*h, and the fp32 h' kept off-chain on GpSimd.
  h state stays fp32; output is written transposed and reassembled on host.
"""

import warnings

warnings.filterwarnings("ignore")

import numpy as np
import ml_dtypes

import concourse.bacc as bacc
import concourse.tile as tile
from concourse import mybir, bass_utils

B, D, H = 64, 512, 512
NCORES = 8
BL = B // NCORES  # batch per core
KD = D // 128  # input-dim k-chunks
KH = H // 128  # hidden-dim k-chunks
M3 = 3 * H // 128  # 3H m-tiles
RT = 8  # output-ring steps per DMA
PF = 2  # gi_rz psum prefill lead (steps)
CW = 512  # precompute chunk width (psum bank = 512 f32)
BF16 = mybir.dt.bfloat16
F32 = mybir.dt.float32
NPBF16 = ml_dtypes.bfloat16

_cache: dict = {}


def _build(T: int, use_bi: bool, use_bhn: bool):
    TB = T * BL
    assert T % RT == 0 and TB % CW == 0
    CH = TB // CW  # precompute chunks
    nc = bacc.Bacc("TRN2", target_bir_lowering=False, debug=False, num_devices=NCORES)

    xt_d = nc.dram_tensor("xt", [128, KD * TB], BF16, kind="ExternalInput").ap()
    wi_d = nc.dram_tensor("wi", [128, M3 * KD * 128], BF16, kind="ExternalInput").ap()
    wh_d = nc.dram_tensor("wh", [128, M3 * KH * 128], BF16, kind="ExternalInput").ap()
    h0_d = nc.dram_tensor("h0t", [128, KH * BL], F32, kind="ExternalInput").ap()
    bi_d = (
        nc.dram_tensor("bi_r", [1, M3 * 128], BF16, kind="ExternalInput").ap()
        if use_bi
        else None
    )
    bhn_d = (
        nc.dram_tensor("bhn_t", [128, KH], F32, kind="ExternalInput").ap()
        if use_bhn
        else None
    )
    ys_d = nc.dram_tensor("yst", [128, KH * TB], F32, kind="ExternalOutput").ap()
    ys_v = ys_d.rearrange("p (k t j) -> p k t j", k=KH, j=BL)

    with tile.TileContext(nc) as tc:
        with (
            tc.tile_pool(name="const", bufs=1) as const,
            tc.tile_pool(name="gib", bufs=1) as gib,
            tc.tile_pool(name="xin", bufs=1) as xin,
            tc.tile_pool(name="pre", bufs=3, space="PSUM") as pre,
            tc.tile_pool(name="whps", bufs=2, space="PSUM") as whps,
            tc.tile_pool(name="gps", bufs=PF + 1, space="PSUM") as gps,
            tc.tile_pool(name="orp", bufs=3) as orp,
            tc.tile_pool(name="hbp", bufs=3) as hbp,
            tc.tile_pool(name="ew", bufs=2) as ew,
        ):
            # ---- load constants ----
            wi_sb = const.tile([128, M3 * KD * 128], BF16)
            nc.sync.dma_start(wi_sb[:], wi_d[:])
            wh_sb = const.tile([128, M3 * KH * 128], BF16)
            nc.sync.dma_start(wh_sb[:], wh_d[:])
            h0_sb = const.tile([128, KH, BL], F32)
            nc.sync.dma_start(h0_sb[:], h0_d.rearrange("p (k j) -> p k j", j=BL))
            if use_bi:
                bi_sb = const.tile([1, M3 * 128], BF16)
                nc.sync.dma_start(bi_sb[:], bi_d[:])
                ones_sb = const.tile([1, CW], BF16)
                nc.vector.memset(ones_sb[:], 1.0)
            if use_bhn:
                bhn_sb = const.tile([128, KH], F32)
                nc.sync.dma_start(bhn_sb[:], bhn_d[:])
            xt_sb = xin.tile([128, KD * TB], BF16)
            nc.sync.dma_start(xt_sb[:], xt_d[:])

            # ---- phase 1: dense gi precompute into SBUF (bf16) ----
            gi_sb = gib.tile([128, M3, TB], BF16)
            for c in range(CH):
                for m in range(M3):
                    pp = pre.tile([128, CW], F32, tag="pre")
                    for k in range(KD):
                        nc.tensor.matmul(
                            pp[:],
                            wi_sb[:, (m * KD + k) * 128 : (m * KD + k + 1) * 128],
                            xt_sb[:, k * TB + c * CW : k * TB + (c + 1) * CW],
                            start=(k == 0),
                            stop=(k == KD - 1) and not use_bi,
                            skip_group_check=True,
                        )
                    if use_bi:
                        nc.tensor.matmul(
                            pp[:],
                            bi_sb[:, m * 128 : (m + 1) * 128],
                            ones_sb[:],
                            start=False,
                            stop=True,
                            skip_group_check=True,
                        )
                    # alternate copy engines so they hide behind the matmuls
                    dst = gi_sb[:, m, c * CW : (c + 1) * CW]
                    if m % 2 == 0:
                        nc.vector.tensor_copy(dst, pp[:])
                    else:
                        nc.scalar.copy(dst, pp[:])

            gi_v = gi_sb.rearrange("p m (t j) -> p m t j", j=BL)

            # ---- phase 2: the scan ----
            def prefill(tt):
                """gi_rz for step tt -> fresh psum bank (off-chain DVE copy).
                gh r/z matmuls later accumulate on top with start=False."""
                g = gps.tile([128, 8, BL], F32, tag="gp")
                nc.vector.tensor_copy(g[:], gi_v[:, 0:8, tt, :])
                return g

            pend = [prefill(tt) for tt in range(min(PF, T))]

            hb = hbp.tile([128, KH, BL], BF16, tag="hb")
            nc.vector.tensor_copy(hb[:], h0_sb[:])
            h_prev = h0_sb[:, :, :]

            o_cur = None
            for t in range(T):
                u = t % RT
                if u == 0:
                    o_cur = orp.tile([128, KH, RT, BL], F32, tag="oring")

                gp = pend.pop(0)
                # on-chain: ghT matmuls (need h from last step).
                # r/z rows accumulate onto the prefilled gi_rz psum; n rows
                # into their own psum so r can gate gh_n alone.
                ps = whps.tile([128, KH, BL], F32, tag="whp")
                for m in range(M3):
                    into_gp = m < 8
                    out_ap = gp[:, m, :] if into_gp else ps[:, m - 8, :]
                    for k in range(KH):
                        nc.tensor.matmul(
                            out_ap,
                            wh_sb[:, (m * KH + k) * 128 : (m * KH + k + 1) * 128],
                            hb[:, k, :],
                            start=(m == 8 and k == 0),
                            stop=(k == KH - 1),
                            skip_group_check=True,
                        )

                # Gate math.  Critical chain (whole-tile ops):
                #   sigmoid(psum) -> rpn -> pre_n -> tanh -> w -> hb
                # Off-chain on GpSimd: omz = 1-z, v = z*h_prev, fp32 h'.
                # h' = (1-z)*n + z*h = omz*n + v
                rzt = ew.tile([128, 8, BL], F32, tag="rzt")
                nc.scalar.activation(
                    rzt[:], gp[:, 0:8, :], mybir.ActivationFunctionType.Sigmoid
                )
                omz = ew.tile([128, KH, BL], F32, tag="omz")
                nc.gpsimd.tensor_scalar(
                    omz[:],
                    rzt[:, KH : 2 * KH, :],
                    -1.0,
                    1.0,
                    mybir.AluOpType.mult,
                    mybir.AluOpType.add,
                )
                v = ew.tile([128, KH, BL], F32, tag="v")
                nc.gpsimd.tensor_mul(v[:], rzt[:, KH : 2 * KH, :], h_prev)
                rpn = ew.tile([128, KH, BL], F32, tag="rpn")
                if use_bhn:
                    for k in range(KH):
                        nc.vector.scalar_tensor_tensor(
                            rpn[:, k, :],
                            ps[:, k, :],
                            bhn_sb[:, k : k + 1],
                            rzt[:, k, :],
                            mybir.AluOpType.add,
                            mybir.AluOpType.mult,
                        )
                else:
                    nc.vector.tensor_mul(rpn[:], ps[:], rzt[:, 0:KH, :])
                pre_n = ew.tile([128, KH, BL], F32, tag="pren")
                nc.vector.tensor_add(pre_n[:], rpn[:], gi_v[:, 8:12, t, :])
                nt = ew.tile([128, KH, BL], F32, tag="nt")
                nc.scalar.activation(
                    nt[:], pre_n[:], mybir.ActivationFunctionType.Tanh
                )
                w = ew.tile([128, KH, BL], F32, tag="w")
                hb = hbp.tile([128, KH, BL], BF16, tag="hb")
                nc.vector.tensor_mul(w[:], nt[:], omz[:])
                nc.vector.tensor_add(hb[:], w[:], v[:])
                h_new = o_cur[:, :, u, :]
                # fp32 h for output/next-step v, off the critical chain
                nc.gpsimd.tensor_add(h_new, w[:], v[:])
                h_prev = h_new

                if t + PF < T:
                    pend.append(prefill(t + PF))

                if u == RT - 1:
                    nc.sync.dma_start(
                        ys_v[:, :, t - RT + 1 : t + 1, :], o_cur[:]
                    )

    nc.compile()
    return nc


def _get(T, use_bi, use_bhn):
    key = (T, use_bi, use_bhn)
    if key not in _cache:
        _cache[key] = _build(T, use_bi, use_bhn)
    return _cache[key]


def _pack_w(W, kc):
    # W [kc*128, M3*128] -> [128, M3*kc*128], col ((m*kc)+k)*128+c = W[k*128+p, m*128+c]
    return np.ascontiguousarray(
        W.astype(NPBF16).reshape(kc, 128, M3, 128).transpose(1, 2, 0, 3).reshape(128, -1)
    )


def kernel(x, h0, Wi, Wh, bi, bhn, _trace=False, _trace_kwargs=None):
    T = x.shape[0]
    use_bi = bool(np.any(bi))
    use_bhn = bool(np.any(bhn))
    nc = _get(T, use_bi, use_bhn)
    TB = T * BL

    wi_p = _pack_w(np.asarray(Wi), KD)
    wh_p = _pack_w(np.asarray(Wh), KH)
    x = np.asarray(x)
    h0 = np.asarray(h0)

    in_maps = []
    for c in range(NCORES):
        xc = x[:, c * BL : (c + 1) * BL, :]  # [T, BL, D]
        xt = np.ascontiguousarray(
            xc.astype(NPBF16).reshape(T, BL, KD, 128).transpose(3, 2, 0, 1).reshape(128, KD * TB)
        )
        h0c = np.ascontiguousarray(
            h0[c * BL : (c + 1) * BL, :].astype(np.float32).reshape(BL, KH, 128).transpose(2, 1, 0).reshape(128, KH * BL)
        )
        im = {"xt": xt, "wi": wi_p, "wh": wh_p, "h0t": h0c}
        if use_bi:
            im["bi_r"] = np.ascontiguousarray(bi.astype(NPBF16).reshape(1, M3 * 128))
        if use_bhn:
            im["bhn_t"] = np.ascontiguousarray(bhn.astype(np.float32).reshape(KH, 128).T)
        in_maps.append(im)

    kw = {}
    if _trace:
        kw = dict(trace=True, **(_trace_kwargs or {}))
    kernel._last_in_maps = in_maps
    res = bass_utils.run_bass_kernel_spmd(nc, in_maps, core_ids=list(range(NCORES)), **kw)

    ys = np.empty((T, B, H), dtype=np.float32)
    for c in range(NCORES):
        out = res.results[c]["yst"]  # [128, KH*TB]
        ys[:, c * BL : (c + 1) * BL, :] = (
            out.reshape(128, KH, T, BL).transpose(2, 3, 1, 0).reshape(T, BL, H)
        )
    kernel._last_result = res
    return ys


# revision 6
# speedup vs baseline: 1.2015x; 1.0255x over previous
"""GRU (Flax GRUCell scanned over time) on 8 Trainium2 NeuronCores.

Problem: x:[T,B,D]=[512,64,512], h0:[B,H], Wi:[D,3H], Wh:[H,3H], bi:[3H], bhn:[H]
  gi = x_t @ Wi + bi ; gh = h @ Wh ; gates (r,z,n); h' = (1-z)*n + z*h
  returns ys:[T,B,H] (the h trajectory).

Strategy (per core, data-parallel over batch, B_local=8):
  Everything on-chip lives in "T-layout": hidden dim on SBUF partitions,
  batch on the free dim.

  Phase 1 (one-time, ~90us): gi = x @ Wi for ALL T steps as dense
  N=512 matmuls (weight loads amortized 64x vs per-step), stored bf16
  in SBUF ([128, M3, T*BL] ~ 98KB/partition).

  Phase 2 (the scan): per step only the 48 gh matmuls run on the PE.
    - the r/z psum bank is PREFILLED with gi_rz by an off-chain DVE
      copy (2 steps ahead); gh r/z matmuls accumulate on top with
      start=False, so sigmoid still reads (gi+gh) straight from PSUM.
    - r/z m-tiles issue first (m-outer) so their accumulation groups
      close early; sigmoid's PE-sem wait overlaps the n-part matmuls.
    - gi_n is read from SBUF by pre_n (it cannot be pre-merged into
      the gh_n psum because r gates only gh_n).
  Gate math critical chain (whole-tile ops):
    sigmoid(psum) -> rpn -> pre_n -> tanh -> w -> hb(bf16)
  with omz = 1-z, v = z*h, and the fp32 h' kept off-chain on GpSimd.
  h state stays fp32; output is written transposed and reassembled on host.
"""

import warnings

warnings.filterwarnings("ignore")

import numpy as np
import ml_dtypes

import concourse.bacc as bacc
import concourse.tile as tile
from concourse import mybir, bass_utils

B, D, H = 64, 512, 512
NCORES = 8
BL = B // NCORES  # batch per core
KD = D // 128  # input-dim k-chunks
KH = H // 128  # hidden-dim k-chunks
M3 = 3 * H // 128  # 3H m-tiles
RT = 8  # output-ring steps per DMA
PF = 2  # gi_rz psum prefill lead (steps)
CW = 512  # precompute chunk width (psum bank = 512 f32)
BF16 = mybir.dt.bfloat16
F16 = mybir.dt.float16
F32 = mybir.dt.float32
NPBF16 = ml_dtypes.bfloat16

_cache: dict = {}


def _build(T: int, use_bi: bool, use_bhn: bool):
    TB = T * BL
    assert T % RT == 0 and TB % CW == 0
    CH = TB // CW  # precompute chunks
    nc = bacc.Bacc("TRN2", target_bir_lowering=False, debug=False, num_devices=NCORES)

    xt_d = nc.dram_tensor("xt", [128, KD * TB], BF16, kind="ExternalInput").ap()
    wi_d = nc.dram_tensor("wi", [128, M3 * KD * 128], BF16, kind="ExternalInput").ap()
    wh_d = nc.dram_tensor("wh", [128, M3 * KH * 128], BF16, kind="ExternalInput").ap()
    h0_d = nc.dram_tensor("h0t", [128, KH * BL], F32, kind="ExternalInput").ap()
    bi_d = (
        nc.dram_tensor("bi_r", [1, M3 * 128], BF16, kind="ExternalInput").ap()
        if use_bi
        else None
    )
    bhn_d = (
        nc.dram_tensor("bhn_t", [128, KH], F32, kind="ExternalInput").ap()
        if use_bhn
        else None
    )
    ys_d = nc.dram_tensor("yst", [128, KH * TB], F32, kind="ExternalOutput").ap()
    ys_v = ys_d.rearrange("p (k t j) -> p k t j", k=KH, j=BL)

    with tile.TileContext(nc) as tc:
        with (
            tc.tile_pool(name="const", bufs=1) as const,
            tc.tile_pool(name="gib", bufs=1) as gib,
            tc.tile_pool(name="xin", bufs=1) as xin,
            tc.tile_pool(name="pre", bufs=3, space="PSUM") as pre,
            tc.tile_pool(name="whps", bufs=2, space="PSUM") as whps,
            tc.tile_pool(name="gps", bufs=PF + 1, space="PSUM") as gps,
            tc.tile_pool(name="orp", bufs=3) as orp,
            tc.tile_pool(name="hbp", bufs=3) as hbp,
            tc.tile_pool(name="ew", bufs=2) as ew,
        ):
            # ---- load constants ----
            wi_sb = const.tile([128, M3 * KD * 128], BF16)
            nc.sync.dma_start(wi_sb[:], wi_d[:])
            wh_sb = const.tile([128, M3 * KH * 128], BF16)
            nc.sync.dma_start(wh_sb[:], wh_d[:])
            h0_sb = const.tile([128, KH, BL], F32)
            nc.sync.dma_start(h0_sb[:], h0_d.rearrange("p (k j) -> p k j", j=BL))
            if use_bi:
                bi_sb = const.tile([1, M3 * 128], BF16)
                nc.sync.dma_start(bi_sb[:], bi_d[:])
                ones_sb = const.tile([1, CW], BF16)
                nc.vector.memset(ones_sb[:], 1.0)
            if use_bhn:
                bhn_sb = const.tile([128, KH], F32)
                nc.sync.dma_start(bhn_sb[:], bhn_d[:])
            xt_sb = xin.tile([128, KD * TB], BF16)
            nc.sync.dma_start(xt_sb[:], xt_d[:])

            # ---- phase 1: dense gi precompute into SBUF (fp16: bf16 would
            # cost ~1e-2 rel err through the recurrence; fp16's 10-bit
            # mantissa keeps the gi rounding ~8x smaller at the same size) ----
            gi_sb = gib.tile([128, M3, TB], F16)
            for c in range(CH):
                for m in range(M3):
                    pp = pre.tile([128, CW], F32, tag="pre")
                    for k in range(KD):
                        nc.tensor.matmul(
                            pp[:],
                            wi_sb[:, (m * KD + k) * 128 : (m * KD + k + 1) * 128],
                            xt_sb[:, k * TB + c * CW : k * TB + (c + 1) * CW],
                            start=(k == 0),
                            stop=(k == KD - 1) and not use_bi,
                            skip_group_check=True,
                        )
                    if use_bi:
                        nc.tensor.matmul(
                            pp[:],
                            bi_sb[:, m * 128 : (m + 1) * 128],
                            ones_sb[:],
                            start=False,
                            stop=True,
                            skip_group_check=True,
                        )
                    # alternate copy engines so they hide behind the matmuls
                    dst = gi_sb[:, m, c * CW : (c + 1) * CW]
                    if m % 2 == 0:
                        nc.vector.tensor_copy(dst, pp[:])
                    else:
                        nc.scalar.copy(dst, pp[:])

            gi_v = gi_sb.rearrange("p m (t j) -> p m t j", j=BL)

            # ---- phase 2: the scan ----
            def prefill(tt):
                """gi_rz for step tt -> fresh psum bank (off-chain DVE copy).
                gh r/z matmuls later accumulate on top with start=False."""
                g = gps.tile([128, 8, BL], F32, tag="gp")
                nc.vector.tensor_copy(g[:], gi_v[:, 0:8, tt, :])
                return g

            pend = [prefill(tt) for tt in range(min(PF, T))]

            hb = hbp.tile([128, KH, BL], BF16, tag="hb")
            nc.vector.tensor_copy(hb[:], h0_sb[:])
            h_prev = h0_sb[:, :, :]

            o_cur = None
            for t in range(T):
                u = t % RT
                if u == 0:
                    o_cur = orp.tile([128, KH, RT, BL], F32, tag="oring")

                gp = pend.pop(0)
                # prefill for t+PF is emitted BEFORE the matmul burst so the
                # DVE CAST lands in the burst window, not mid-gate-chain
                if t + PF < T:
                    pend.append(prefill(t + PF))

                # on-chain: ghT matmuls (need h from last step).
                # r/z rows accumulate onto the prefilled gi_rz psum; n rows
                # into their own psum so r can gate gh_n alone.
                # Issue k0..1 (ready with the hb low half) for all m first,
                # then k2..3: the first 24 matmuls overlap the hb tail.
                ps = whps.tile([128, KH, BL], F32, tag="whp")
                for khalf in range(2):
                    for m in range(M3):
                        out_ap = gp[:, m, :] if m < 8 else ps[:, m - 8, :]
                        for k in (2 * khalf, 2 * khalf + 1):
                            nc.tensor.matmul(
                                out_ap,
                                wh_sb[:, (m * KH + k) * 128 : (m * KH + k + 1) * 128],
                                hb[:, k, :],
                                start=(m == 8 and k == 0),
                                stop=(k == KH - 1),
                                skip_group_check=True,
                            )

                # Gate math.  Critical chain (whole-tile ops):
                #   sigmoid(psum) -> rpn -> pre_n -> tanh -> w -> hb
                # sigmoid is split r-then-z: rpn only needs r, so it starts
                # ~120ns earlier; z feeds only the off-chain omz/v.
                # Off-chain on GpSimd: omz = 1-z, v = z*h_prev, fp32 h'.
                # h' = (1-z)*n + z*h = omz*n + v
                rzt = ew.tile([128, 8, BL], F32, tag="rzt")
                nc.scalar.activation(
                    rzt[:, 0:KH, :], gp[:, 0:KH, :],
                    mybir.ActivationFunctionType.Sigmoid,
                )
                nc.scalar.activation(
                    rzt[:, KH : 2 * KH, :], gp[:, KH : 2 * KH, :],
                    mybir.ActivationFunctionType.Sigmoid,
                )
                omz = ew.tile([128, KH, BL], F32, tag="omz")
                nc.gpsimd.tensor_scalar(
                    omz[:],
                    rzt[:, KH : 2 * KH, :],
                    -1.0,
                    1.0,
                    mybir.AluOpType.mult,
                    mybir.AluOpType.add,
                )
                v = ew.tile([128, KH, BL], F32, tag="v")
                nc.gpsimd.tensor_mul(v[:], rzt[:, KH : 2 * KH, :], h_prev)
                rpn = ew.tile([128, KH, BL], F32, tag="rpn")
                if use_bhn:
                    for k in range(KH):
                        nc.vector.scalar_tensor_tensor(
                            rpn[:, k, :],
                            ps[:, k, :],
                            bhn_sb[:, k : k + 1],
                            rzt[:, k, :],
                            mybir.AluOpType.add,
                            mybir.AluOpType.mult,
                        )
                else:
                    nc.vector.tensor_mul(rpn[:], ps[:], rzt[:, 0:KH, :])
                pre_n = ew.tile([128, KH, BL], F32, tag="pren")
                nc.vector.tensor_add(pre_n[:], rpn[:], gi_v[:, 8:12, t, :])
                nt = ew.tile([128, KH, BL], F32, tag="nt")
                nc.scalar.activation(
                    nt[:], pre_n[:], mybir.ActivationFunctionType.Tanh
                )
                # tail in halves: hb low half (k0..1) lands ~200ns before the
                # high half, releasing the next step's k0..1 matmuls early
                w = ew.tile([128, KH, BL], F32, tag="w")
                hb = hbp.tile([128, KH, BL], BF16, tag="hb")
                nc.vector.tensor_mul(w[:, 0:2, :], nt[:, 0:2, :], omz[:, 0:2, :])
                nc.vector.tensor_mul(w[:, 2:4, :], nt[:, 2:4, :], omz[:, 2:4, :])
                nc.vector.tensor_add(hb[:, 0:2, :], w[:, 0:2, :], v[:, 0:2, :])
                nc.vector.tensor_add(hb[:, 2:4, :], w[:, 2:4, :], v[:, 2:4, :])
                h_new = o_cur[:, :, u, :]
                # fp32 h for output/next-step v, off the critical chain
                nc.gpsimd.tensor_add(h_new, w[:], v[:])
                h_prev = h_new

                if u == RT - 1:
                    nc.sync.dma_start(
                        ys_v[:, :, t - RT + 1 : t + 1, :], o_cur[:]
                    )

    nc.compile()
    return nc


def _get(T, use_bi, use_bhn):
    key = (T, use_bi, use_bhn)
    if key not in _cache:
        _cache[key] = _build(T, use_bi, use_bhn)
    return _cache[key]


def _pack_w(W, kc):
    # W [kc*128, M3*128] -> [128, M3*kc*128], col ((m*kc)+k)*128+c = W[k*128+p, m*128+c]
    return np.ascontiguousarray(
        W.astype(NPBF16).reshape(kc, 128, M3, 128).transpose(1, 2, 0, 3).reshape(128, -1)
    )


def kernel(x, h0, Wi, Wh, bi, bhn, _trace=False, _trace_kwargs=None):
    T = x.shape[0]
    use_bi = bool(np.any(bi))
    use_bhn = bool(np.any(bhn))
    nc = _get(T, use_bi, use_bhn)
    TB = T * BL

    wi_p = _pack_w(np.asarray(Wi), KD)
    wh_p = _pack_w(np.asarray(Wh), KH)
    x = np.asarray(x)
    h0 = np.asarray(h0)

    in_maps = []
    for c in range(NCORES):
        xc = x[:, c * BL : (c + 1) * BL, :]  # [T, BL, D]
        xt = np.ascontiguousarray(
            xc.astype(NPBF16).reshape(T, BL, KD, 128).transpose(3, 2, 0, 1).reshape(128, KD * TB)
        )
        h0c = np.ascontiguousarray(
            h0[c * BL : (c + 1) * BL, :].astype(np.float32).reshape(BL, KH, 128).transpose(2, 1, 0).reshape(128, KH * BL)
        )
        im = {"xt": xt, "wi": wi_p, "wh": wh_p, "h0t": h0c}
        if use_bi:
            im["bi_r"] = np.ascontiguousarray(bi.astype(NPBF16).reshape(1, M3 * 128))
        if use_bhn:
            im["bhn_t"] = np.ascontiguousarray(bhn.astype(np.float32).reshape(KH, 128).T)
        in_maps.append(im)

    kw = {}
    if _trace:
        kw = dict(trace=True, **(_trace_kwargs or {}))
    kernel._last_in_maps = in_maps
    res = bass_utils.run_bass_kernel_spmd(nc, in_maps, core_ids=list(range(NCORES)), **kw)

    ys = np.empty((T, B, H), dtype=np.float32)
    for c in range(NCORES):
        out = res.results[c]["yst"]  # [128, KH*TB]
        ys[:, c * BL : (c + 1) * BL, :] = (
            out.reshape(128, KH, T, BL).transpose(2, 3, 1, 0).reshape(T, BL, H)
        )
    kernel._last_result = res
    return ys
